# revision 1
# baseline (speedup 1.0000x reference)
"""Trainium2 Bass kernel for a ViT-style transformer block.

Reference computation (per batch element):
    h   = LN1(x);  qkv = h @ qkv_w.T + [q_bias, 0, v_bias]
    q,k,v per head (16 heads x 64);  attn = softmax(q*scale @ k.T + rel_bias)
    o   = (attn @ v) @ proj_w.T + proj_b;  x2 = x + o
    out = x2 + fc2(gelu(fc1(LN2(x2))))

Strategy: data-parallel over batch across 8 NeuronCores (8 samples each).
Per core, activations flow token-major through LN/residual (bn_stats +
per-partition tensor_scalar) and feature-major through the matmul chain
(weights pre-transposed on host, bf16).  The two layout switches (post-LN)
are bf16 DMA transposes.  Attention computes scores transposed
([keys, queries]) so exp needs no cross-partition max (scores ~ N(0,1);
exp never overflows fp32/bf16), and the attn@v matmul appends a ones
column to V so softmax denominators fall out of the same matmul; the
normalize is then a per-partition reciprocal+multiply in token-major.
"""

import sys

sys.path.insert(0, "/opt/trn_rl_repo")

import numpy as np
import ml_dtypes

import concourse.bass as bass
import concourse.tile as tile
from concourse import mybir
from concourse.vector_clock import ScopedClock
from concourse.bass_utils import run_bass_kernel_spmd

F32 = mybir.dt.float32
BF16 = mybir.dt.bfloat16
AF = mybir.ActivationFunctionType
ALU = mybir.AluOpType

# Problem constants (hardcoded per spec)
B, N_TOK, D = 64, 197, 1024
NCORES = 8
BL = B // NCORES            # samples per core = 8
T = BL * N_TOK              # tokens per core = 1576
NH, HD = 16, 64             # heads
HID = 4096                  # MLP hidden
SCALE = HD ** -0.5
WH = WW = 14
NUM_REL = (2 * WH - 1) * (2 * WW - 1) + 3
LN_EPS = 1e-5

NT = (T + 127) // 128       # 13 token tiles
LASTP = T - 128 * (NT - 1)  # 40
KTP = [128, N_TOK - 128]    # per-sample key tile sizes [128, 69]
# token chunks for N-dim of feature-major matmuls
CHUNKS = [(i * 512, min(512, T - i * 512)) for i in range((T + 511) // 512)]
# fc2 token-tile groups (PSUM-bank limited)
FC2_GROUPS = [(0, 5), (5, 4), (9, 4)]
TPAD = T + 24              # transpose dst padding (16-row xbar granularity)

def _ceil16(n):
    return (n + 15) // 16 * 16


def _tok_tiles():
    """13 token tiles: (start, rows)."""
    return [(t * 128, 128 if t < NT - 1 else LASTP) for t in range(NT)]


def _sample_tiles():
    """16 sample-aligned key tiles: (b, kt, start_token, rows)."""
    out = []
    for b in range(BL):
        for kt in range(2):
            out.append((b, kt, b * N_TOK + kt * 128, KTP[kt]))
    return out


def _make_rel_pos_index():
    coords = np.stack(np.meshgrid(np.arange(WH), np.arange(WW), indexing="ij"))
    flat = coords.reshape(2, -1)
    rel = flat[:, :, None] - flat[:, None, :]
    rel = rel.transpose(1, 2, 0).copy()
    rel[:, :, 0] += WH - 1
    rel[:, :, 1] += WW - 1
    rel[:, :, 0] *= 2 * WW - 1
    idx = np.zeros((N_TOK, N_TOK), dtype=np.int32)
    idx[1:, 1:] = rel.sum(-1)
    idx[0, 0:] = NUM_REL - 3
    idx[0:, 0] = NUM_REL - 2
    idx[0, 0] = NUM_REL - 1
    return idx


class SplitDrainTileContext(tile.TileContext):
    """Walrus in this toolchain rejects >1 sync-wait on the kernel-tail
    Drain; split the waits across a chain of drain instructions."""

    def _drain_and_barrier(self, tick_clock, wait_clock):
        drain_inst = self.nc.sync.drain()
        wait_clock.add_sem_waits(
            drain_inst.ins, ScopedClock({None: tick_clock.global_clock})
        )
        si = drain_inst.ins.sync_info
        waits = list(si.on_wait) if si and si.on_wait else []
        if len(waits) > 1:
            si.on_wait = waits[:1]
            for w in waits[1:]:
                d2 = self.nc.sync.drain()
                si2 = d2.ins.sync_info
                if si2 is None:
                    d2.ins.sync_info = mybir.SyncInfo(on_wait=[w], on_update=[])
                else:
                    si2.on_wait = [w]
        self.nc.all_engine_barrier()
        assert self.sems is not None
        popped = self.nc._tile_sem_poison_stack.pop()
        assert popped is self._sem_poison
        self.nc.clear_and_free_semaphores(list(self.sems.allocated().values()))
        self.nc.all_engine_barrier()


def _layernorm_to_bf16(nc, pool, x_ap, p, eps_tile, out_bf16):
    """Token-major LN: x_ap [p,1024] f32 -> out_bf16 [p,1024] bf16."""
    stats = pool.tile([128, 2, 6], F32, tag="ln_stats", name="ln_stats")
    for sg in range(2):
        nc.vector.bn_stats(out=stats[:p, sg, :], in_=x_ap[:, sg * 512:(sg + 1) * 512])
    mv = pool.tile([128, 2], F32, tag="ln_mv", name="ln_mv")
    nc.vector.bn_aggr(out=mv[:p, :], in_=stats[:p, :, :])
    rstd = pool.tile([128, 1], F32, tag="ln_rstd", name="ln_rstd")
    nc.scalar.activation(out=rstd[:p, :], in_=mv[:p, 1:2], func=AF.Sqrt,
                         bias=eps_tile[:p, :], scale=1.0)
    nc.vector.reciprocal(out=rstd[:p, :], in_=rstd[:p, :])
    nc.vector.tensor_scalar(
        out=out_bf16[:p, :], in0=x_ap, scalar1=mv[:p, 0:1], scalar2=rstd[:p, :],
        op0=ALU.subtract, op1=ALU.mult)



def _split_sync_waits(nc, cap=1):
    """Walrus in this toolchain caps sync-wait commands per instruction; hoist
    excess waits onto standalone event-semaphore instructions inserted just
    before the over-limit instruction on the same engine."""
    n = 0
    for fn in nc.m.functions:
        for bb in fn.blocks:
            insts = bb.instructions
            i = 0
            while i < len(insts):
                inst = insts[i]
                si = inst.sync_info
                waits = list(si.on_wait) if si and si.on_wait else []
                if len(waits) > cap and inst.engine != mybir.EngineType.Unassigned:
                    excess = waits[:len(waits) - cap]
                    si.on_wait = waits[len(waits) - cap:]
                    for w in excess:
                        ev = mybir.InstEventSemaphore(
                            name=f"waitsplit_{n}", ins=[], outs=[],
                            sync_info=mybir.SyncInfo(on_wait=[w], on_update=[]))
                        ev.engine = inst.engine
                        nc.register_instruction(ev)
                        insts.insert(i, ev)
                        n += 1
                        i += 1
                i += 1
    return n


def build_program(debug_taps=False):
    nc = bass.Bass("TRN2", target_bir_lowering=False, debug=False,
                   num_devices=NCORES)

    # ---- DRAM I/O ----
    x_h = nc.declare_dram_parameter("x", [T, D], F32, isOutput=False)
    xres1_h = nc.declare_dram_parameter("xres1", [T, D], F32, isOutput=False)
    qkvwT_h = nc.declare_dram_parameter("qkvwT", [D, 3 * D], BF16, isOutput=False)
    qb_h = nc.declare_dram_parameter("qb", [128, 16], F32, isOutput=False)
    vbrep_h = nc.declare_dram_parameter("vb_rep", [128, D], F32, isOutput=False)
    biasT_h = nc.declare_dram_parameter("biasT", [N_TOK, NH, N_TOK], BF16, isOutput=False)
    projwT_h = nc.declare_dram_parameter("projwT", [D, D], BF16, isOutput=False)
    fc1wT_h = nc.declare_dram_parameter("fc1wT", [32, 128, D], BF16, isOutput=False)
    fc1b_h = nc.declare_dram_parameter("fc1b", [128, 32], F32, isOutput=False)
    fc2wT_h = nc.declare_dram_parameter("fc2wT", [HID, D], BF16, isOutput=False)
    fc2brep_h = nc.declare_dram_parameter("fc2b_rep", [128, D], F32, isOutput=False)
    out_h = nc.declare_dram_parameter("out", [T, D], F32, isOutput=True)
    x2s_h = nc.dram_tensor("x2s", [T, D], F32)  # x2 + fc2_b scratch
    taps = {}
    if debug_taps:
        taps["d_h1T"] = nc.declare_dram_parameter("d_h1T", [128, 8, TPAD], BF16, isOutput=True)
        taps["d_qk"] = nc.declare_dram_parameter("d_qk", [16, 128, T], BF16, isOutput=True)
        taps["d_v"] = nc.declare_dram_parameter("d_v", [16, 128, NH, 65], BF16, isOutput=True)
        taps["d_P"] = nc.declare_dram_parameter("d_P", [2, 128, NH, N_TOK], BF16, isOutput=True)
        taps["d_ctxT"] = nc.declare_dram_parameter("d_ctxT", [8, 128, TPAD], BF16, isOutput=True)
        taps["d_x2"] = nc.declare_dram_parameter("d_x2", [T, D], F32, isOutput=True)
        taps["d_gT"] = nc.declare_dram_parameter("d_gT", [128, 32, T], BF16, isOutput=True)

    tok_tiles = _tok_tiles()
    samp_tiles = _sample_tiles()

    with SplitDrainTileContext(nc) as tc:
        # Pool stack (LIFO release order):
        #   consts -> mid(ctxT,h2T) -> actA(qkt,vt) -> h1Tp -> [phase-scoped]
        consts_cm = tc.tile_pool(name="consts", bufs=1)
        consts = consts_cm.__enter__()
        eps_t = consts.tile([128, 1], F32, tag="eps", name="eps")
        nc.vector.memset(eps_t, LN_EPS)
        qb_t = consts.tile([128, 16], F32, tag="qb", name="qb")
        nc.sync.dma_start(out=qb_t, in_=qb_h[:, :])
        vbrep_t = consts.tile([128, D], F32, tag="vbrep", name="vbrep")
        nc.sync.dma_start(out=vbrep_t, in_=vbrep_h[:, :])

        mid_cm = tc.tile_pool(name="mid", bufs=1)
        midp = mid_cm.__enter__()
        ctxT = [midp.tile([128, TPAD], BF16, tag=f"ctxT{ft}", name=f"ctxT{ft}")
                for ft in range(8)]
        h2T = midp.tile([128, 8, TPAD], BF16, tag="h2T", name="h2T")

        actA_cm = tc.tile_pool(name="actA", bufs=1)
        actA = actA_cm.__enter__()
        qkt = [actA.tile([128, T], BF16, tag=f"qkt{ft}", name=f"qkt{ft}")
               for ft in range(16)]
        vt = {}
        for (b, kt, t0, kp) in samp_tiles:
            vt[(b, kt)] = actA.tile([128, NH, 65], BF16, tag=f"v{b}_{kt}",
                                    name=f"v{b}_{kt}")

        h1T_cm = tc.tile_pool(name="h1Tp", bufs=1)
        h1Tp = h1T_cm.__enter__()
        h1T = h1Tp.tile([128, 8, TPAD], BF16, tag="h1T", name="h1T")

        # ---------- Phase A: LN1 + transpose ----------
        with tc.tile_pool(name="ln1", bufs=3) as ln1p:
            for (t0, p) in tok_tiles:
                xt = ln1p.tile([128, D], F32, tag="x_in", name="x_in")
                nc.sync.dma_start(out=xt[:p, :], in_=x_h[t0:t0 + p, :])
                h1 = ln1p.tile([128, D], BF16, tag="h1", name="h1")
                p16 = _ceil16(p)
                if p16 > p:
                    nc.vector.memset(h1[(p // 32) * 32:p16, :], 0.0)
                _layernorm_to_bf16(nc, ln1p, xt[:p, :], p, eps_t, h1)
                for kt in range(8):
                    nc.sync.dma_start(
                        out=h1T[:, kt, t0:t0 + p16],
                        in_=h1[:p16, kt * 128:(kt + 1) * 128],
                        transpose=True)

        if debug_taps:
            nc.sync.dma_start(out=taps["d_h1T"][:, :, :], in_=h1T[:, :, :])

        # ---------- Phase B: QKV matmuls (V, then Q, then K) ----------
        # V token-major (sample-aligned): out [tok<=128, 512 vfeat]
        with tc.tile_pool(name="vw", bufs=1) as vwp, \
             tc.tile_pool(name="v_ps", bufs=4, space="PSUM") as vps_pool:
            vw = [vwp.tile([128, D], BF16, tag=f"vw{kt}", name=f"vw{kt}")
                  for kt in range(8)]
            for kt in range(8):
                nc.sync.dma_start(out=vw[kt],
                                  in_=qkvwT_h[kt * 128:(kt + 1) * 128, 2 * D:3 * D])
            for (b, kt, t0, kp) in samp_tiles:
                vtile = vt[(b, kt)]
                nc.vector.memset(vtile[:, :, 64:65], 1.0)
                for vc in range(2):
                    ps = vps_pool.tile([128, 512], F32, tag="vps", name="vps")
                    for dk in range(8):
                        nc.tensor.matmul(
                            ps[:kp, :],
                            lhsT=h1T[:, dk, t0:t0 + kp],
                            rhs=vw[dk][:, vc * 512:(vc + 1) * 512],
                            start=(dk == 0), stop=(dk == 7))
                    nc.vector.tensor_add(
                        out=vtile[:kp, vc * 8:(vc + 1) * 8, 0:64],
                        in0=ps[:kp, :].rearrange("p (a d) -> p a d", a=8),
                        in1=vbrep_t[:kp, vc * 512:(vc + 1) * 512].rearrange(
                            "p (a d) -> p a d", a=8))

        # Q then K feature-major: out [feat 128, tok chunk]
        for half in range(2):
            with tc.tile_pool(name=f"qkw{half}", bufs=1) as qkwp, \
                 tc.tile_pool(name=f"qk_ps{half}", bufs=4, space="PSUM") as qkps:
                wq = [qkwp.tile([128, D], BF16, tag=f"qkw{kt}", name=f"qkw{kt}")
                      for kt in range(8)]
                for kt in range(8):
                    nc.sync.dma_start(
                        out=wq[kt],
                        in_=qkvwT_h[kt * 128:(kt + 1) * 128, half * D:(half + 1) * D])
                for fi in range(8):
                    ft = half * 8 + fi
                    for (c0, cw) in CHUNKS:
                        ps = qkps.tile([128, 512], F32, tag="qkps", name="qkps")
                        for kt in range(8):
                            nc.tensor.matmul(
                                ps[:, :cw],
                                lhsT=wq[kt][:, fi * 128:(fi + 1) * 128],
                                rhs=h1T[:, kt, c0:c0 + cw],
                                start=(kt == 0), stop=(kt == 7))
                        nc.scalar.activation(
                            out=qkt[ft][:, c0:c0 + cw], in_=ps[:, :cw],
                            func=AF.Identity, bias=qb_t[:, ft:ft + 1], scale=1.0)

        h1T_cm.__exit__(None, None, None)

        if debug_taps:
            for ft in range(16):
                nc.sync.dma_start(out=taps["d_qk"][ft], in_=qkt[ft][:, :])
            for si, (b, kt, t0, kp) in enumerate(samp_tiles):
                nc.sync.dma_start(out=taps["d_v"][si], in_=vt[(b, kt)][:, :, :])

        # ---------- Phase C: attention ----------
        with tc.tile_pool(name="attn_sb", bufs=1) as attp, \
             tc.tile_pool(name="p_pool", bufs=3) as ppool, \
             tc.tile_pool(name="ctx_sb", bufs=4) as ctxp, \
             tc.tile_pool(name="sc_ps", bufs=1, space="PSUM") as scps, \
             tc.tile_pool(name="ctx_ps", bufs=1, space="PSUM") as ctxps:

            bT = []
            for kt in range(2):
                t_ = attp.tile([128, NH, N_TOK], BF16, tag=f"biasT{kt}",
                               name=f"biasT{kt}")
                kp = KTP[kt]
                nc.sync.dma_start(out=t_[:kp, :, :],
                                  in_=biasT_h[kt * 128: kt * 128 + kp, :, :])
                bT.append(t_)

            pt = {}
            for b in range(BL):
                q0 = b * N_TOK
                # scores^T + exp, per key-tile, 4 heads per PSUM group
                for kt in range(2):
                    kp = KTP[kt]
                    k0 = q0 + kt * 128
                    ptile = ppool.tile([128, NH, N_TOK], BF16, tag="P", name="P")
                    pt[(b, kt)] = ptile
                    for g in range(4):
                        ps = scps.tile([128, 4, 512], F32, tag="scps", name="scps")
                        for gi in range(4):
                            h = g * 4 + gi
                            ft = h // 2
                            rb = (h % 2) * 64
                            nc.tensor.matmul(
                                ps[:kp, gi, 0:N_TOK],
                                lhsT=qkt[8 + ft][rb:rb + 64, k0:k0 + kp],
                                rhs=qkt[ft][rb:rb + 64, q0:q0 + N_TOK],
                                start=True, stop=True)
                        psl = ptile[:kp, g * 4:(g + 1) * 4, :]
                        nc.vector.tensor_add(
                            out=psl,
                            in0=ps[:kp, :, 0:N_TOK],
                            in1=bT[kt][:kp, g * 4:(g + 1) * 4, :])
                        nc.scalar.activation(out=psl, in_=psl, func=AF.Exp)
                    if debug_taps and b == 0:
                        nc.sync.dma_start(out=taps["d_P"][kt], in_=ptile[:, :, :])

                # ctx token-major with fused sumexp (ones column of V)
                for qt in range(2):
                    qn = KTP[qt]
                    qoff = qt * 128
                    ps = ctxps.tile([128, NH, 128], F32, tag="ctxps", name="ctxps")
                    for h in range(NH):
                        for kt in range(2):
                            kp = KTP[kt]
                            nc.tensor.matmul(
                                ps[:qn, h, 0:65],
                                lhsT=pt[(b, kt)][:kp, h, qoff:qoff + qn],
                                rhs=vt[(b, kt)][:kp, h, :],
                                start=(kt == 0), stop=(kt == 1))
                    rec = ctxp.tile([128, NH], F32, tag="rec", name="rec")
                    nc.vector.reciprocal(out=rec[:qn, :], in_=ps[:qn, :, 64])
                    cs = ctxp.tile([128, NH, HD], BF16, tag="ctx", name="ctx")
                    qn16 = _ceil16(qn)
                    if qn16 > qn:
                        nc.vector.memset(cs[(qn // 32) * 32:qn16, :, :], 0.0)
                    nc.vector.tensor_mul(
                        out=cs[:qn, :, :],
                        in0=ps[:qn, :, 0:64],
                        in1=rec[:qn, :, None].broadcast_to([qn, NH, HD]))
                    # transpose to feature-major ctxT via a 16-col-aligned
                    # staging tile (xbar transpose writes in 16-element column
                    # units; ctxT's per-sample offsets b*197 are unaligned),
                    # then plain-DMA exactly qn valid columns into place.
                    stage = ctxp.tile([128, 8, 128], BF16, tag="ctx_stage",
                                      name="ctx_stage")
                    for blk in range(8):
                        nc.sync.dma_start(
                            out=stage[:, blk, 0:qn16],
                            in_=cs[:qn16, blk * 2:blk * 2 + 2, :],
                            transpose=True)
                    c0 = b * N_TOK + qoff
                    for blk in range(8):
                        nc.sync.dma_start(
                            out=ctxT[blk][:, c0:c0 + qn],
                            in_=stage[:, blk, 0:qn])

        if debug_taps:
            for ft in range(8):
                nc.sync.dma_start(out=taps["d_ctxT"][ft], in_=ctxT[ft][:, :])

        actA_cm.__exit__(None, None, None)

        lateC_cm = tc.tile_pool(name="lateC", bufs=1)
        lateC = lateC_cm.__enter__()
        fc1b_t = lateC.tile([128, 32], F32, tag="fc1b", name="fc1b")
        nc.sync.dma_start(out=fc1b_t, in_=fc1b_h[:, :])
        fc2brep_t = lateC.tile([128, D], F32, tag="fc2brep", name="fc2brep")
        nc.sync.dma_start(out=fc2brep_t, in_=fc2brep_h[:, :])

        # ---------- Phase D: proj + residual + LN2 ----------
        with tc.tile_pool(name="projw", bufs=1) as projwp, \
             tc.tile_pool(name="proj_ps", bufs=2, space="PSUM") as projps, \
             tc.tile_pool(name="proj_sb", bufs=3) as projsb:
            pw = [projwp.tile([128, D], BF16, tag=f"projw{kt}", name=f"projw{kt}")
                  for kt in range(8)]
            for kt in range(8):
                nc.sync.dma_start(out=pw[kt], in_=projwT_h[kt * 128:(kt + 1) * 128, :])
            for (t0, p) in tok_tiles:
                xr = projsb.tile([128, D], F32, tag="xres", name="xres")
                nc.sync.dma_start(out=xr[:p, :], in_=xres1_h[t0:t0 + p, :])
                x2 = projsb.tile([128, D], F32, tag="x2", name="x2")
                for f in range(2):
                    ps = projps.tile([128, 512], F32, tag="projps", name="projps")
                    for kt in range(8):
                        nc.tensor.matmul(
                            ps[:p, :],
                            lhsT=ctxT[kt][:, t0:t0 + p],
                            rhs=pw[kt][:, f * 512:(f + 1) * 512],
                            start=(kt == 0), stop=(kt == 7))
                    nc.vector.tensor_add(
                        out=x2[:p, f * 512:(f + 1) * 512],
                        in0=ps[:p, :], in1=xr[:p, f * 512:(f + 1) * 512])
                # x2 + fc2_b -> HBM scratch (residual base for fc2 drain)
                if debug_taps:
                    nc.sync.dma_start(out=taps["d_x2"][t0:t0 + p, :], in_=x2[:p, :])
                x2fb = projsb.tile([128, D], F32, tag="x2fb", name="x2fb")
                nc.vector.tensor_add(out=x2fb[:p, :], in0=x2[:p, :],
                                     in1=fc2brep_t[:p, :])
                nc.sync.dma_start(out=x2s_h[t0:t0 + p, :], in_=x2fb[:p, :])
                # LN2 -> h2 bf16 -> transpose
                h2 = projsb.tile([128, D], BF16, tag="h2", name="h2")
                p16 = _ceil16(p)
                if p16 > p:
                    nc.vector.memset(h2[(p // 32) * 32:p16, :], 0.0)
                _layernorm_to_bf16(nc, projsb, x2[:p, :], p, eps_t, h2)
                for kt in range(8):
                    nc.sync.dma_start(
                        out=h2T[:, kt, t0:t0 + p16],
                        in_=h2[:p16, kt * 128:(kt + 1) * 128],
                        transpose=True)

        # ---------- Phase E: MLP ----------
        gT_cm = tc.tile_pool(name="gT_pool", bufs=1)
        gTp = gT_cm.__enter__()
        gT = gTp.tile([128, 32, T], BF16, tag="gT", name="gT")
        with tc.tile_pool(name="fc1w", bufs=3) as fc1wp, \
             tc.tile_pool(name="fc1_ps", bufs=2, space="PSUM") as fc1ps:
            for Ht in range(32):
                wt = fc1wp.tile([128, D], BF16, tag="fc1w", name="fc1w")
                nc.sync.dma_start(out=wt, in_=fc1wT_h[Ht, :, :])
                for (c0, cw) in CHUNKS:
                    ps = fc1ps.tile([128, 512], F32, tag="fc1ps", name="fc1ps")
                    for kt in range(8):
                        nc.tensor.matmul(
                            ps[:, :cw],
                            lhsT=wt[:, kt * 128:(kt + 1) * 128],
                            rhs=h2T[:, kt, c0:c0 + cw],
                            start=(kt == 0), stop=(kt == 7))
                    nc.scalar.activation(
                        out=gT[:, Ht, c0:c0 + cw], in_=ps[:, :cw],
                        func=AF.Gelu, bias=fc1b_t[:, Ht:Ht + 1], scale=1.0)

        if debug_taps:
            nc.sync.dma_start(out=taps["d_gT"][:, :, :], in_=gT[:, :, :])

        with tc.tile_pool(name="fc2w", bufs=3) as fc2wp, \
             tc.tile_pool(name="fc2_ps", bufs=5, space="PSUM") as fc2ps, \
             tc.tile_pool(name="fc2_sb", bufs=6) as fc2sb:
            for f in range(2):
                for (g0, gn) in FC2_GROUPS:
                    pss = [fc2ps.tile([128, 512], F32, tag="fc2ps", name="fc2ps")
                           for _ in range(gn)]
                    for Hkt in range(32):
                        w2 = fc2wp.tile([128, 512], BF16, tag="fc2w", name="fc2w")
                        nc.sync.dma_start(
                            out=w2,
                            in_=fc2wT_h[Hkt * 128:(Hkt + 1) * 128,
                                        f * 512:(f + 1) * 512])
                        for i in range(gn):
                            t0, p = tok_tiles[g0 + i]
                            nc.tensor.matmul(
                                pss[i][:p, :],
                                lhsT=gT[:, Hkt, t0:t0 + p],
                                rhs=w2,
                                start=(Hkt == 0), stop=(Hkt == 31))
                    for i in range(gn):
                        t0, p = tok_tiles[g0 + i]
                        xf = fc2sb.tile([128, 512], F32, tag="x2fb_in", name="x2fb_in")
                        nc.sync.dma_start(
                            out=xf[:p, :],
                            in_=x2s_h[t0:t0 + p, f * 512:(f + 1) * 512])
                        ot = fc2sb.tile([128, 512], F32, tag="out_sb", name="out_sb")
                        nc.vector.tensor_add(out=ot[:p, :], in0=pss[i][:p, :],
                                             in1=xf[:p, :])
                        nc.sync.dma_start(
                            out=out_h[t0:t0 + p, f * 512:(f + 1) * 512],
                            in_=ot[:p, :])
        gT_cm.__exit__(None, None, None)
        lateC_cm.__exit__(None, None, None)
        mid_cm.__exit__(None, None, None)
        consts_cm.__exit__(None, None, None)
    _split_sync_waits(nc)
    return nc


_CACHED_NC = None


def _get_nc():
    global _CACHED_NC
    if _CACHED_NC is None:
        _CACHED_NC = build_program()
    return _CACHED_NC


def prepare_host_inputs(x, qkv_w, q_bias, v_bias, rel_bias_table, proj_w, proj_b,
                        ln1_g, ln1_b, ln2_g, ln2_b, fc1_w, fc1_b, fc2_w, fc2_b):
    """Fold LN affine params / scale into weights; pre-transpose; gather
    rel-pos bias; build the per-core input maps."""
    bf = ml_dtypes.bfloat16
    f32 = np.float32
    x = np.asarray(x, f32)

    # fold LN1 gamma/beta into qkv weights, scale q by 1/8
    qkv_b = np.concatenate([q_bias, np.zeros_like(v_bias), v_bias]).astype(f32)
    W1 = qkv_w.astype(f32) * ln1_g[None, :].astype(f32)
    b1 = qkv_b + qkv_w.astype(f32) @ ln1_b.astype(f32)
    W1[:D] *= SCALE
    b1[:D] *= SCALE
    qkvwT = np.ascontiguousarray(W1.T).astype(bf)          # [1024, 3072]
    qb = np.ascontiguousarray(b1[:2 * D].reshape(16, 128).T).astype(f32)  # [128,16]
    vb_rep = np.broadcast_to(b1[2 * D:], (128, D)).copy().astype(f32)

    # rel-pos bias, transposed to [k, h, q]
    idx = _make_rel_pos_index()
    rel = rel_bias_table.astype(f32)[idx]                  # [q, k, h]
    biasT = np.ascontiguousarray(rel.transpose(1, 2, 0)).astype(bf)  # [k, h, q]

    projwT = np.ascontiguousarray(proj_w.astype(f32).T).astype(bf)    # [1024,1024]

    # fold LN2 gamma/beta into fc1
    W3 = fc1_w.astype(f32) * ln2_g[None, :].astype(f32)
    b3 = fc1_b.astype(f32) + fc1_w.astype(f32) @ ln2_b.astype(f32)
    W3T = np.ascontiguousarray(W3.T)                       # [1024, 4096]
    # tile layout [32 Ht, 128 k? -> [Ht][p][kt*128+m]
    fc1wT = W3T.reshape(8, 128, 32, 128).transpose(2, 1, 0, 3)
    fc1wT = np.ascontiguousarray(fc1wT.reshape(32, 128, D)).astype(bf)
    fc1b = np.ascontiguousarray(b3.reshape(32, 128).T).astype(f32)    # [128,32]

    fc2wT = np.ascontiguousarray(fc2_w.astype(f32).T).astype(bf)      # [4096,1024]
    fc2b_rep = np.broadcast_to(fc2_b.astype(f32), (128, D)).copy()

    xres1 = x + proj_b[None, None, :].astype(f32)

    shared = dict(qkvwT=qkvwT, qb=qb, vb_rep=vb_rep, biasT=biasT,
                  projwT=projwT, fc1wT=fc1wT, fc1b=fc1b, fc2wT=fc2wT,
                  fc2b_rep=fc2b_rep)
    in_maps = []
    for c in range(NCORES):
        sl = slice(c * BL, (c + 1) * BL)
        m = dict(shared)
        m["x"] = np.ascontiguousarray(x[sl].reshape(T, D))
        m["xres1"] = np.ascontiguousarray(xres1[sl].reshape(T, D))
        in_maps.append(m)
    return in_maps


def kernel(**inputs):
    nc = _get_nc()
    in_maps = prepare_host_inputs(**inputs)
    res = run_bass_kernel_spmd(nc, in_maps, list(range(NCORES)))
    outs = [res.results[c]["out"].reshape(BL, N_TOK, D) for c in range(NCORES)]
    return np.concatenate(outs, axis=0).astype(np.float32)

